# revision 22
# baseline (speedup 1.0000x reference)
"""Trainium2 Bass kernel for nn_LinearNet (complex double-linear).

Reference math (N = 4096):
    R_r = x @ W_r^T          R_i = x @ W_i^T
    C_r = W_r^T @ R_r - W_i^T @ R_i
    C_i = W_r^T @ R_i + W_i^T @ R_r
    out = concat([C_r, C_i], axis=1)                    # [N, 2N]

Sharding: core c owns output columns S_c = [c*512, (c+1)*512) of both C_r
and C_i.  Pass 1 computes R[:, S_c] = x @ W[S_c, :]^T; R stays resident in
SBUF (bf16).  Pass 2 uses Gauss's 3-multiplication complex product:
    m1 = W_r^T (R_r + R_i)
    m2 = (W_r + W_i)^T R_i
    m3 = (W_r - W_i)^T R_r
    C_r = m1 - m2,  C_i = m1 - m3
so only 5 matmul-units total instead of 6.  All matmuls are bf16 with
fp32 PSUM accumulation (single-pass PE streaming + fast weight load,
~213ns per 128x512 matmul vs ~466ns for fp32).  W_r+W_i / W_r-W_i are
precomputed on the host.  No inter-core communication.
"""

import numpy as np
import ml_dtypes

BF16 = ml_dtypes.bfloat16

N = 4096
P = 128
NCORES = 8
SH = N // NCORES  # 512 output columns per core
KT = N // P  # 32 contraction tiles
NSUP = 256  # pass-2 output-row super tile (2 PSUM triples live)

_CACHE = {}


def _build():
    import concourse.mybir as mybir
    import concourse.tile as tile
    from concourse import bacc

    f32 = mybir.dt.float32
    bf16 = mybir.dt.bfloat16
    ADD = mybir.AluOpType.add
    SUB = mybir.AluOpType.subtract

    nc = bacc.Bacc()
    # packed inputs (host layouts, see kernel()):
    #  xpk : [16ms, 16kp, 128, 512] -> rows (ms*16+kp)*128, k-pair packed in free
    #  wtpk: [32k, 128, 1024] -> concat(wrT[k], wiT[k]) in free dim
    #  wpk : [16a, 32j, 128, 768] -> concat(wr, ws, wd)[j-block, a-cols(256)]
    xpk = nc.declare_dram_parameter("xpk", [16 * 16 * P, 512], bf16, isOutput=False)
    wtpk = nc.declare_dram_parameter("wtpk", [KT * P, 1024], bf16, isOutput=False)
    wpk = nc.declare_dram_parameter("wpk", [16 * KT * P, 768], bf16, isOutput=False)
    out_r = nc.declare_dram_parameter("out_r", [N, SH], f32, isOutput=True)
    out_i = nc.declare_dram_parameter("out_i", [N, SH], f32, isOutput=True)

    with tile.TileContext(nc) as tc:
        with (
            tc.tile_pool(name="rbig", bufs=1) as rbig,
            tc.tile_pool(name="wt", bufs=1) as wt_pool,
            tc.tile_pool(name="xs", bufs=6) as xs_pool,
            tc.tile_pool(name="wv", bufs=8) as wv_pool,
            tc.tile_pool(name="m1s", bufs=4) as m1s_pool,
            tc.tile_pool(name="outp", bufs=4) as outp_pool,
        ):
            # R resident in SBUF for the whole kernel (bf16, 32KB/partition each)
            rr_sb = rbig.tile([P, KT * SH], bf16)
            ri_sb = rbig.tile([P, KT * SH], bf16)
            rs_sb = rbig.tile([P, KT * SH], bf16)

            # ---------- pass 1: R[:, S_c] = x @ W[S_c, :]^T ----------
            wt_sb = wt_pool.tile([P, KT * 1024], bf16)
            for k in range(KT):
                nc.scalar.dma_start(
                    wt_sb[:, k * 1024 : (k + 1) * 1024],
                    wtpk[k * P : (k + 1) * P, :],
                )

            # first super-tile covers 512 output rows (msub=4, 8 MMs per
            # k-step) so the PE consumes pass-1 weight tiles at half rate
            # while the scalar queue is still delivering them; the rest use
            # msub=2 (4 PSUM banks live, room to rotate)
            supers = [(0, 4)] + [(4 + 2 * i, 2) for i in range(14)]
            with tc.tile_pool(name="ps1", bufs=4, space="PSUM") as ps1:
                for ms_start, msub in supers:
                    acc_r = [
                        ps1.tile([P, SH], f32, tag="ps_r", name=f"accr{_s}")
                        for _s in range(msub)
                    ]
                    acc_i = [
                        ps1.tile([P, SH], f32, tag="ps_i", name=f"acci{_s}")
                        for _s in range(msub)
                    ]
                    for kp in range(KT // 2):  # 16 k-pairs
                        xcs = []
                        for half in range(msub // 2):
                            xc = xs_pool.tile([P, 512], bf16, tag="xc")
                            row = ((ms_start // 2 + half) * 16 + kp) * P
                            nc.sync.dma_start(xc[:], xpk[row : row + P, :])
                            xcs.append(xc)
                        for kh in range(2):
                            k = 2 * kp + kh
                            first, last = k == 0, k == KT - 1
                            for s in range(msub):
                                xc = xcs[s // 2]
                                ss = s % 2
                                lhs = xc[
                                    :, kh * 256 + ss * P : kh * 256 + (ss + 1) * P
                                ]
                                nc.tensor.matmul(
                                    acc_r[s][:],
                                    lhs,
                                    wt_sb[:, k * 1024 : k * 1024 + 512],
                                    start=first,
                                    stop=last,
                                )
                                nc.tensor.matmul(
                                    acc_i[s][:],
                                    lhs,
                                    wt_sb[:, k * 1024 + 512 : (k + 1) * 1024],
                                    start=first,
                                    stop=last,
                                )
                    for s in range(msub):
                        mt = ms_start + s
                        sl = slice(mt * SH, (mt + 1) * SH)
                        # alternate engines per sub-tile so consecutive bank
                        # evictions run in parallel on vector+scalar: the
                        # next super's PSUM slot reuse (and the pass-1 ->
                        # pass-2 pool handoff) waits half as long
                        if s % 2 == 0:
                            nc.vector.tensor_copy(rr_sb[:, sl], acc_r[s][:])
                            nc.scalar.copy(ri_sb[:, sl], acc_i[s][:])
                        else:
                            nc.scalar.copy(rr_sb[:, sl], acc_r[s][:])
                            nc.vector.tensor_copy(ri_sb[:, sl], acc_i[s][:])
                        nc.vector.tensor_tensor(
                            rs_sb[:, sl], rr_sb[:, sl], ri_sb[:, sl], ADD
                        )

            # ---------- pass 2: Gauss 3-mult complex product ----------
            asub = NSUP // P  # 2
            with tc.tile_pool(name="ps2", bufs=2, space="PSUM") as ps2:
                for a in range(N // NSUP):  # 16
                    pm1 = [
                        ps2.tile([P, SH], f32, tag="m1", bufs=4, name=f"pm1{_s}")
                        for _s in range(asub)
                    ]
                    pm2 = [
                        ps2.tile([P, SH], f32, tag="m2", bufs=2, name=f"pm2{_s}")
                        for _s in range(asub)
                    ]
                    pm3 = [
                        ps2.tile([P, SH], f32, tag="m3", bufs=2, name=f"pm3{_s}")
                        for _s in range(asub)
                    ]
                    for j in range(KT):  # 32
                        wv = wv_pool.tile([P, 768], bf16, tag="wv")
                        row = (a * KT + j) * P
                        if j % 2 == 0:
                            nc.sync.dma_start(wv[:], wpk[row : row + P, :])
                        else:
                            nc.scalar.dma_start(wv[:], wpk[row : row + P, :])
                        rrj = rr_sb[:, j * SH : (j + 1) * SH]
                        rij = ri_sb[:, j * SH : (j + 1) * SH]
                        rsj = rs_sb[:, j * SH : (j + 1) * SH]
                        first, last = j == 0, j == KT - 1
                        for s in range(asub):
                            sl = slice(s * P, (s + 1) * P)
                            nc.tensor.matmul(
                                pm1[s][:], wv[:, 0:256][:, sl], rsj,
                                start=first, stop=last,
                            )
                        for s in range(asub):
                            sl = slice(s * P, (s + 1) * P)
                            nc.tensor.matmul(
                                pm2[s][:], wv[:, 256:512][:, sl], rij,
                                start=first, stop=last,
                            )
                            nc.tensor.matmul(
                                pm3[s][:], wv[:, 512:768][:, sl], rrj,
                                start=first, stop=last,
                            )
                    for s in range(asub):
                        at = a * asub + s
                        m1c = m1s_pool.tile([P, SH], f32, tag="m1c")
                        nc.scalar.copy(m1c[:], pm1[s][:])
                        ocr = outp_pool.tile([P, SH], f32, tag="ocr")
                        oci = outp_pool.tile([P, SH], f32, tag="oci")
                        nc.vector.tensor_tensor(ocr[:], m1c[:], pm2[s][:], SUB)
                        nc.vector.tensor_tensor(oci[:], m1c[:], pm3[s][:], SUB)
                        # final a-super rides the (idle, HWDGE) sync queue so
                        # the kernel tail isn't gated by the slow SWDGE queue
                        oq = nc.sync if a == N // NSUP - 1 else nc.gpsimd
                        oq.dma_start(out_r[at * P : (at + 1) * P, :], ocr[:])
                        oq.dma_start(out_i[at * P : (at + 1) * P, :], oci[:])

    nc.finalize()
    return nc


def _get_nc():
    if "nc" not in _CACHE:
        _CACHE["nc"] = _build()
    return _CACHE["nc"]


def _pack_x(xb):
    # xb: bf16 [N, N] (natural x layout).  Want tiles of xT = x.T:
    # xT[k*128:(k+1)*128, ms*256:(ms+1)*256] packed as
    # xpk[ms, kp, 128, 512] with k-pair (k=2kp, 2kp+1) side by side.
    t = np.ascontiguousarray(xb.T).reshape(KT, P, 16, 256)
    t = np.ascontiguousarray(t.transpose(2, 0, 1, 3))  # [16ms, 32k, 128, 256]
    t = t.reshape(16, 16, 2, P, 256).transpose(0, 1, 3, 2, 4)  # [ms, kp, 128, 2, 256]
    return np.ascontiguousarray(t).reshape(16 * 16 * P, 512)


def _pack_wt(wrb, wib, sl):
    # per-core pass-1 weights: concat(wrT[k], wiT[k]) along free dim
    out = np.empty((KT, P, 1024), dtype=BF16)
    out[:, :, 0:512] = np.ascontiguousarray(wrb[sl].T).reshape(KT, P, SH)
    out[:, :, 512:1024] = np.ascontiguousarray(wib[sl].T).reshape(KT, P, SH)
    return out.reshape(KT * P, 1024)


def _pack_w2(wrb, wsb, wdb):
    # pass-2 stream: wpk[a, j, 128, 768] = concat(wr, ws, wd)[j-block, a-cols]
    out = np.empty((16, KT, P, 768), dtype=BF16)
    for idx, V in enumerate((wrb, wsb, wdb)):
        t = V.reshape(KT, P, 16, 256).transpose(2, 0, 1, 3)  # [16a, 32j, 128, 256]
        out[:, :, :, idx * 256 : (idx + 1) * 256] = t
    return out.reshape(16 * KT * P, 768)


def kernel(x, W_r, W_i, **run_kwargs):
    from concourse.bass_utils import run_bass_kernel_spmd

    x = np.asarray(x, dtype=np.float32)
    W_r = np.asarray(W_r, dtype=np.float32)
    W_i = np.asarray(W_i, dtype=np.float32)

    nc = _get_nc()

    xb = x.astype(BF16)
    wrb = W_r.astype(BF16)
    wib = W_i.astype(BF16)
    wsb = (W_r + W_i).astype(BF16)
    wdb = (W_r - W_i).astype(BF16)

    xpk = _pack_x(xb)
    wpk = _pack_w2(wrb, wsb, wdb)

    in_maps = []
    for c in range(NCORES):
        sl = slice(c * SH, (c + 1) * SH)
        in_maps.append(
            {
                "xpk": xpk,
                "wtpk": _pack_wt(wrb, wib, sl),
                "wpk": wpk,
            }
        )
    out = run_bass_kernel_spmd(nc, in_maps, list(range(NCORES)), **run_kwargs)
    res = out.results

    full = np.empty((N, 2 * N), dtype=np.float32)
    for c in range(NCORES):
        full[:, c * SH : (c + 1) * SH] = res[c]["out_r"]
        full[:, N + c * SH : N + (c + 1) * SH] = res[c]["out_i"]
    if run_kwargs:
        _CACHE["last_result"] = out
    return full



# revision 24
# speedup vs baseline: 1.0031x; 1.0031x over previous
"""Trainium2 Bass kernel for nn_LinearNet (complex double-linear).

Reference math (N = 4096):
    R_r = x @ W_r^T          R_i = x @ W_i^T
    C_r = W_r^T @ R_r - W_i^T @ R_i
    C_i = W_r^T @ R_i + W_i^T @ R_r
    out = concat([C_r, C_i], axis=1)                    # [N, 2N]

Sharding: core c owns output columns S_c = [c*512, (c+1)*512) of both C_r
and C_i.  Pass 1 computes R[:, S_c] = x @ W[S_c, :]^T; R stays resident in
SBUF (bf16).  Pass 2 uses Gauss's 3-multiplication complex product:
    m1 = W_r^T (R_r + R_i)
    m2 = (W_r + W_i)^T R_i
    m3 = (W_r - W_i)^T R_r
    C_r = m1 - m2,  C_i = m1 - m3
so only 5 matmul-units total instead of 6.  All matmuls are bf16 with
fp32 PSUM accumulation (single-pass PE streaming + fast weight load,
~213ns per 128x512 matmul vs ~466ns for fp32).  W_r+W_i / W_r-W_i are
precomputed on the host.  No inter-core communication.
"""

import numpy as np
import ml_dtypes

BF16 = ml_dtypes.bfloat16

N = 4096
P = 128
NCORES = 8
SH = N // NCORES  # 512 output columns per core
KT = N // P  # 32 contraction tiles
NSUP = 256  # pass-2 output-row super tile (2 PSUM triples live)

_CACHE = {}


def _build():
    import concourse.mybir as mybir
    import concourse.tile as tile
    from concourse import bacc

    f32 = mybir.dt.float32
    bf16 = mybir.dt.bfloat16
    ADD = mybir.AluOpType.add
    SUB = mybir.AluOpType.subtract

    nc = bacc.Bacc()
    # packed inputs (host layouts, see kernel()):
    #  xpk : [16ms, 16kp, 128, 512] -> rows (ms*16+kp)*128, k-pair packed in free
    #  wtpk: [32k, 128, 1024] -> concat(wrT[k], wiT[k]) in free dim
    #  wpk : [16a, 32j, 128, 768] -> concat(wr, ws, wd)[j-block, a-cols(256)]
    xpk = nc.declare_dram_parameter("xpk", [16 * 16 * P, 512], bf16, isOutput=False)
    wtpk = nc.declare_dram_parameter("wtpk", [KT * P, 1024], bf16, isOutput=False)
    wpk = nc.declare_dram_parameter("wpk", [16 * KT * P, 768], bf16, isOutput=False)
    out_r = nc.declare_dram_parameter("out_r", [N, SH], f32, isOutput=True)
    out_i = nc.declare_dram_parameter("out_i", [N, SH], f32, isOutput=True)

    with tile.TileContext(nc) as tc:
        with (
            tc.tile_pool(name="rbig", bufs=1) as rbig,
            tc.tile_pool(name="wt", bufs=1) as wt_pool,
            tc.tile_pool(name="xs", bufs=6) as xs_pool,
            tc.tile_pool(name="wv", bufs=8) as wv_pool,
            tc.tile_pool(name="m1s", bufs=4) as m1s_pool,
            tc.tile_pool(name="outp", bufs=4) as outp_pool,
        ):
            # R resident in SBUF for the whole kernel (bf16, 32KB/partition each)
            rr_sb = rbig.tile([P, KT * SH], bf16)
            ri_sb = rbig.tile([P, KT * SH], bf16)
            rs_sb = rbig.tile([P, KT * SH], bf16)

            # ---------- pass 1: R[:, S_c] = x @ W[S_c, :]^T ----------
            wt_sb = wt_pool.tile([P, KT * 1024], bf16)
            for k in range(KT):
                nc.scalar.dma_start(
                    wt_sb[:, k * 1024 : (k + 1) * 1024],
                    wtpk[k * P : (k + 1) * P, :],
                )

            # first super-tile covers 512 output rows (msub=4, 8 MMs per
            # k-step) so the PE consumes pass-1 weight tiles at half rate
            # while the scalar queue is still delivering them; the rest use
            # msub=2 (4 PSUM banks live, room to rotate)
            supers = [(0, 4)] + [(4 + 2 * i, 2) for i in range(14)]
            with tc.tile_pool(name="ps1", bufs=4, space="PSUM") as ps1:
                # HAM pre-warm: the PE sits idle from preamble end (~6us)
                # until the first DMA-fed matmul (~10.4us), and the first
                # ~3.4us of real matmuls would otherwise run at the cold
                # 1.2GHz clock.  16 dummy matmuls on a zeroed tile fill the
                # dead window and un-throttle the clock gate for free.
                # 4+4 PSUM allocations keep both tag rings aligned mod 4;
                # the real accumulation groups overwrite via start=True.
                warm = xs_pool.tile([P, 512], bf16, tag="warm", bufs=1, name="warm")
                nc.vector.memset(warm[:], 0.0)
                for t in range(4):
                    wmr = ps1.tile([P, SH], f32, tag="ps_r", name=f"warmr{t}")
                    wmi = ps1.tile([P, SH], f32, tag="ps_i", name=f"warmi{t}")
                    for wt_ in (wmr, wmi):
                        nc.tensor.matmul(
                            wt_[:], warm[:, 0:P], warm[:], start=True, stop=False
                        )
                        nc.tensor.matmul(
                            wt_[:], warm[:, 0:P], warm[:], start=False, stop=True
                        )
                for ms_start, msub in supers:
                    acc_r = [
                        ps1.tile([P, SH], f32, tag="ps_r", name=f"accr{_s}")
                        for _s in range(msub)
                    ]
                    acc_i = [
                        ps1.tile([P, SH], f32, tag="ps_i", name=f"acci{_s}")
                        for _s in range(msub)
                    ]
                    for kp in range(KT // 2):  # 16 k-pairs
                        xcs = []
                        for half in range(msub // 2):
                            xc = xs_pool.tile([P, 512], bf16, tag="xc")
                            row = ((ms_start // 2 + half) * 16 + kp) * P
                            nc.sync.dma_start(xc[:], xpk[row : row + P, :])
                            xcs.append(xc)
                        for kh in range(2):
                            k = 2 * kp + kh
                            first, last = k == 0, k == KT - 1
                            for s in range(msub):
                                xc = xcs[s // 2]
                                ss = s % 2
                                lhs = xc[
                                    :, kh * 256 + ss * P : kh * 256 + (ss + 1) * P
                                ]
                                nc.tensor.matmul(
                                    acc_r[s][:],
                                    lhs,
                                    wt_sb[:, k * 1024 : k * 1024 + 512],
                                    start=first,
                                    stop=last,
                                )
                                nc.tensor.matmul(
                                    acc_i[s][:],
                                    lhs,
                                    wt_sb[:, k * 1024 + 512 : (k + 1) * 1024],
                                    start=first,
                                    stop=last,
                                )
                    for s in range(msub):
                        mt = ms_start + s
                        sl = slice(mt * SH, (mt + 1) * SH)
                        # split the two PSUM reads across vector+scalar so the
                        # bank-release fence (and the pass-1 -> pass-2 PSUM
                        # pool handoff) is half as long
                        nc.vector.tensor_copy(rr_sb[:, sl], acc_r[s][:])
                        nc.scalar.copy(ri_sb[:, sl], acc_i[s][:])
                        nc.vector.tensor_tensor(
                            rs_sb[:, sl], rr_sb[:, sl], ri_sb[:, sl], ADD
                        )

            # ---------- pass 2: Gauss 3-mult complex product ----------
            asub = NSUP // P  # 2
            with tc.tile_pool(name="ps2", bufs=2, space="PSUM") as ps2:
                for a in range(N // NSUP):  # 16
                    pm1 = [
                        ps2.tile([P, SH], f32, tag="m1", bufs=4, name=f"pm1{_s}")
                        for _s in range(asub)
                    ]
                    pm2 = [
                        ps2.tile([P, SH], f32, tag="m2", bufs=2, name=f"pm2{_s}")
                        for _s in range(asub)
                    ]
                    pm3 = [
                        ps2.tile([P, SH], f32, tag="m3", bufs=2, name=f"pm3{_s}")
                        for _s in range(asub)
                    ]
                    for j in range(KT):  # 32
                        wv = wv_pool.tile([P, 768], bf16, tag="wv")
                        row = (a * KT + j) * P
                        if j % 2 == 0:
                            nc.sync.dma_start(wv[:], wpk[row : row + P, :])
                        else:
                            nc.scalar.dma_start(wv[:], wpk[row : row + P, :])
                        rrj = rr_sb[:, j * SH : (j + 1) * SH]
                        rij = ri_sb[:, j * SH : (j + 1) * SH]
                        rsj = rs_sb[:, j * SH : (j + 1) * SH]
                        first, last = j == 0, j == KT - 1
                        for s in range(asub):
                            sl = slice(s * P, (s + 1) * P)
                            nc.tensor.matmul(
                                pm1[s][:], wv[:, 0:256][:, sl], rsj,
                                start=first, stop=last,
                            )
                        for s in range(asub):
                            sl = slice(s * P, (s + 1) * P)
                            nc.tensor.matmul(
                                pm2[s][:], wv[:, 256:512][:, sl], rij,
                                start=first, stop=last,
                            )
                            nc.tensor.matmul(
                                pm3[s][:], wv[:, 512:768][:, sl], rrj,
                                start=first, stop=last,
                            )
                    for s in range(asub):
                        at = a * asub + s
                        m1c = m1s_pool.tile([P, SH], f32, tag="m1c")
                        nc.scalar.copy(m1c[:], pm1[s][:])
                        ocr = outp_pool.tile([P, SH], f32, tag="ocr")
                        oci = outp_pool.tile([P, SH], f32, tag="oci")
                        nc.vector.tensor_tensor(ocr[:], m1c[:], pm2[s][:], SUB)
                        nc.vector.tensor_tensor(oci[:], m1c[:], pm3[s][:], SUB)
                        # final a-super rides the (idle, HWDGE) sync queue so
                        # the kernel tail isn't gated by the slow SWDGE queue
                        oq = nc.sync if a == N // NSUP - 1 else nc.gpsimd
                        oq.dma_start(out_r[at * P : (at + 1) * P, :], ocr[:])
                        oq.dma_start(out_i[at * P : (at + 1) * P, :], oci[:])

    nc.finalize()
    return nc


def _get_nc():
    if "nc" not in _CACHE:
        _CACHE["nc"] = _build()
    return _CACHE["nc"]


def _pack_x(xb):
    # xb: bf16 [N, N] (natural x layout).  Want tiles of xT = x.T:
    # xT[k*128:(k+1)*128, ms*256:(ms+1)*256] packed as
    # xpk[ms, kp, 128, 512] with k-pair (k=2kp, 2kp+1) side by side.
    t = np.ascontiguousarray(xb.T).reshape(KT, P, 16, 256)
    t = np.ascontiguousarray(t.transpose(2, 0, 1, 3))  # [16ms, 32k, 128, 256]
    t = t.reshape(16, 16, 2, P, 256).transpose(0, 1, 3, 2, 4)  # [ms, kp, 128, 2, 256]
    return np.ascontiguousarray(t).reshape(16 * 16 * P, 512)


def _pack_wt(wrb, wib, sl):
    # per-core pass-1 weights: concat(wrT[k], wiT[k]) along free dim
    out = np.empty((KT, P, 1024), dtype=BF16)
    out[:, :, 0:512] = np.ascontiguousarray(wrb[sl].T).reshape(KT, P, SH)
    out[:, :, 512:1024] = np.ascontiguousarray(wib[sl].T).reshape(KT, P, SH)
    return out.reshape(KT * P, 1024)


def _pack_w2(wrb, wsb, wdb):
    # pass-2 stream: wpk[a, j, 128, 768] = concat(wr, ws, wd)[j-block, a-cols]
    out = np.empty((16, KT, P, 768), dtype=BF16)
    for idx, V in enumerate((wrb, wsb, wdb)):
        t = V.reshape(KT, P, 16, 256).transpose(2, 0, 1, 3)  # [16a, 32j, 128, 256]
        out[:, :, :, idx * 256 : (idx + 1) * 256] = t
    return out.reshape(16 * KT * P, 768)


def kernel(x, W_r, W_i, **run_kwargs):
    from concourse.bass_utils import run_bass_kernel_spmd

    x = np.asarray(x, dtype=np.float32)
    W_r = np.asarray(W_r, dtype=np.float32)
    W_i = np.asarray(W_i, dtype=np.float32)

    nc = _get_nc()

    xb = x.astype(BF16)
    wrb = W_r.astype(BF16)
    wib = W_i.astype(BF16)
    wsb = (W_r + W_i).astype(BF16)
    wdb = (W_r - W_i).astype(BF16)

    xpk = _pack_x(xb)
    wpk = _pack_w2(wrb, wsb, wdb)

    in_maps = []
    for c in range(NCORES):
        sl = slice(c * SH, (c + 1) * SH)
        in_maps.append(
            {
                "xpk": xpk,
                "wtpk": _pack_wt(wrb, wib, sl),
                "wpk": wpk,
            }
        )
    out = run_bass_kernel_spmd(nc, in_maps, list(range(NCORES)), **run_kwargs)
    res = out.results

    full = np.empty((N, 2 * N), dtype=np.float32)
    for c in range(NCORES):
        full[:, c * SH : (c + 1) * SH] = res[c]["out_r"]
        full[:, N + c * SH : N + (c + 1) * SH] = res[c]["out_i"]
    if run_kwargs:
        _CACHE["last_result"] = out
    return full

